# revision 10
# baseline (speedup 1.0000x reference)
"""ChannelAttention Trainium2 kernel.

Per-sample computation (B=8, one sample per NeuronCore):
    K   = x[b] viewed as (C=64, N=110592)   (raw row-major reshape)
    G   = K @ K^T                            (64, 64)
    m3  = G @ G
    A   = sigmoid(m3)                        (fully saturated 0/1 in practice)
    out = gamma * (A @ K) + x[b]

Kernel structure per core (memory-bound):
  Phase A: stream K as natural (128, 2048) fp32 tiles (two 2048-column
    chunks stacked on partition halves -> full-width DMA), cast each on
    GpSimd to a resident bf16 copy (13.8 MB, fits SBUF).
  Phase B: hardware xbar DMA-transpose of each resident bf16 slice
    (64, 2048) -> (128, 16, 64): sixteen K^T chunks per op, no
    per-element descriptors. Each (128, 64) slice feeds an accumulating
    PE matmul -> G in PSUM (bf16 Gram is safe: sigmoid(G@G) saturates
    to exact 0/1; measured absmax output impact 5e-4).
  Middle: m3 = G@G duplicated into both PSUM partition halves, sigmoid
    on ACT, scale by gamma (folded so the phase-C epilogue is one DVE op).
  Phase C: re-read the fp32 tiles, per-half matmul against the
    gamma-scaled affinity (concurrent PE quadrants (0,0)/(64,64)),
    epilogue out = W_psum + x_tile in a single DVE add, store.
"""

from contextlib import ExitStack

import numpy as np

import concourse.bass as bass
import concourse.mybir as mybir
import concourse.tile as tile
from concourse.bass_utils import run_bass_kernel_spmd


def split_waits(nc, max_waits: int = 1) -> int:
    """Walrus workaround: cayman instruction structs have a single
    NEURON_ISA_TPB_EVENTS slot and this walrus build rejects BIR
    instructions carrying more sync waits ("Too many sync wait commands").
    Move excess waits onto no-fuse NoOps inserted immediately before the
    instruction on the same engine — per-engine program order preserves the
    blocking semantics. Updates are never moved."""
    n_inserted = 0
    for f in nc.m.functions:
        for b in f.blocks:
            out = []
            changed = False
            for inst in b.instructions:
                si = inst.sync_info
                waits = list(si.on_wait) if si and si.on_wait else []
                if len(waits) > max_waits:
                    changed = True
                    si.on_wait = waits[:max_waits]
                    rest = waits[max_waits:]
                    for k in range(0, len(rest), max_waits):
                        n_inserted += 1
                        nop = mybir.InstNoOp(
                            name=f"I-waitsplit-{n_inserted}",
                            engine=inst.engine,
                            ins=[],
                            outs=[],
                            bass_nofuse=True,
                            sync_info=mybir.SyncInfo(
                                on_wait=rest[k : k + max_waits], on_update=[]
                            ),
                        )
                        nc.register_instruction(nop, overwrite=True)
                        out.append(nop)
                out.append(inst)
            if changed:
                b.instructions = out
    return n_inserted


F32 = mybir.dt.float32
BF16 = mybir.dt.bfloat16

B = 8
C = 64
N = 48 * 48 * 48  # 110592
P = 128

NT = 2048  # columns per stacked half; load tile = (128, NT) fp32 = 1 MiB
TRJ = NT // P  # 16 transposed (128, 64) chunks per xbar op
MM_N = 512  # fp32 moving-operand max free dim


def build_channel_attention(n: int = N) -> bass.Bass:
    n_tiles = n // (2 * NT)  # tiles of (128, NT) covering 2*NT columns each
    assert n_tiles * 2 * NT == n

    nc = bass.Bass()
    x_d = nc.dram_tensor("x", [C, n], F32, kind="ExternalInput")
    g_d = nc.dram_tensor("gamma", [1, 1], F32, kind="ExternalInput")
    o_d = nc.dram_tensor("out", [C, n], F32, kind="ExternalOutput")

    with tile.TileContext(nc) as tc, ExitStack() as ctx:
        singles = ctx.enter_context(tc.tile_pool(name="singles", bufs=1))
        xin = ctx.enter_context(tc.tile_pool(name="xin", bufs=5))
        rk = ctx.enter_context(tc.tile_pool(name="rk", bufs=n_tiles))
        trp = ctx.enter_context(tc.tile_pool(name="trp", bufs=3))
        oout = ctx.enter_context(tc.tile_pool(name="oout", bufs=3))
        gram_ps = ctx.enter_context(tc.tile_pool(name="gram_ps", bufs=1, space="PSUM"))
        w_ps_pool = ctx.enter_context(tc.tile_pool(name="w_ps", bufs=3, space="PSUM"))

        # gamma broadcast to all 128 partitions (per-partition scalar operand)
        gamma_sb = singles.tile([P, 1], F32)
        nc.gpsimd.dma_start(
            out=gamma_sb,
            in_=bass.AP(tensor=g_d, offset=0, ap=[[0, P], [1, 1]]),
        )

        # (n_chunks, C, NT) view of K: xn[u, c, f] = K[c, u*NT + f]
        xn = x_d[:, :].rearrange("c (t f) -> t c f", f=NT)
        on = o_d[:, :].rearrange("c (t f) -> t c f", f=NT)

        # ---------------- Phase A+B: resident bf16 cast + Gram ----------------
        gram = gram_ps.tile([C, C], F32)
        rks = []
        mm_i = 0
        n_gram_mms = n_tiles * 2 * TRJ
        for t in range(n_tiles):
            xt = xin.tile([P, NT], F32, tag="xin")
            nc.sync.dma_start(out=xt, in_=xn[2 * t : 2 * t + 2])
            rkt = rk.tile([P, NT], BF16, tag="rk")
            nc.gpsimd.tensor_copy(rkt, xt)
            rks.append(rkt)
            for a in range(2):
                tr = trp.tile([P, TRJ, C], BF16, tag="trp")
                nc.scalar.dma_start_transpose(tr, rkt[a * C : (a + 1) * C, :])
                for j in range(TRJ):
                    nc.tensor.matmul(
                        gram,
                        tr[:, j, :],
                        tr[:, j, :],
                        start=(mm_i == 0),
                        stop=(mm_i == n_gram_mms - 1),
                    )
                    mm_i += 1

        # ---------------- Middle: A = gamma * sigmoid(G @ G) ----------------
        g_sb = singles.tile([C, C], F32)
        nc.vector.tensor_copy(g_sb, gram)
        m3 = gram_ps.tile([P, C], F32)
        # duplicate m3 into both partition halves so phase C can use
        # both PE row groups (stacked rhs tiles)
        nc.tensor.matmul(m3[0:C, :], g_sb, g_sb, start=True, stop=True)
        nc.tensor.matmul(m3[C:P, :], g_sb, g_sb, start=True, stop=True)
        a2 = singles.tile([P, C], F32)
        zero_bias = singles.tile([P, 1], F32)
        nc.vector.memset(zero_bias, 0.0)
        nc.scalar.activation(
            a2, m3, mybir.ActivationFunctionType.Sigmoid, bias=zero_bias
        )
        nc.vector.tensor_scalar_mul(a2, a2, gamma_sb)

        # ---------------- Phase C: out = (gamma*A) @ K + x ----------------
        for t in range(n_tiles):
            xt = xin.tile([P, NT], F32, tag="xin")
            nc.sync.dma_start(out=xt, in_=xn[2 * t : 2 * t + 2])
            ot = oout.tile([P, NT], F32)
            for f0 in range(0, NT, MM_N):
                w_ps = w_ps_pool.tile([P, MM_N], F32)
                nc.tensor.matmul(
                    w_ps[0:C, :],
                    a2[0:C, :],
                    xt[0:C, f0 : f0 + MM_N],
                    start=True,
                    stop=True,
                )
                nc.tensor.matmul(
                    w_ps[C:P, :],
                    a2[C:P, :],
                    xt[C:P, f0 : f0 + MM_N],
                    start=True,
                    stop=True,
                )
                nc.vector.tensor_add(
                    ot[:, f0 : f0 + MM_N], w_ps, xt[:, f0 : f0 + MM_N]
                )
            nc.scalar.dma_start(out=on[2 * t : 2 * t + 2], in_=ot)

    split_waits(nc)
    return nc


_NC_CACHE: dict[int, bass.Bass] = {}


def _get_nc(n: int = N) -> bass.Bass:
    if n not in _NC_CACHE:
        _NC_CACHE[n] = build_channel_attention(n)
    return _NC_CACHE[n]


def kernel(x: np.ndarray, gamma: np.ndarray, **run_kwargs):
    x = np.ascontiguousarray(np.asarray(x, dtype=np.float32))
    b = x.shape[0]
    n = int(np.prod(x.shape[1:])) // C
    g11 = np.asarray(gamma, dtype=np.float32).reshape(1, 1)
    xs = x.reshape(b, C, n)

    nc = _get_nc(n)
    in_maps = [{"x": xs[i], "gamma": g11} for i in range(b)]
    res = run_bass_kernel_spmd(nc, in_maps, core_ids=list(range(b)), **run_kwargs)
    out = np.stack([res.results[i]["out"] for i in range(b)])
    if run_kwargs.get("trace"):
        kernel.last_result = res
    return out.reshape(x.shape).astype(np.float32)


# revision 12
# speedup vs baseline: 1.4024x; 1.4024x over previous
"""ChannelAttention Trainium2 kernel.

Per-sample computation (B=8, one sample per NeuronCore):
    K   = x[b] viewed as (C=64, N=110592)   (raw row-major reshape)
    G   = K @ K^T                            (64, 64)
    m3  = G @ G
    A   = sigmoid(m3)                        (fully saturated 0/1 in practice)
    out = gamma * (A @ K) + x[b]

Kernel structure per core (memory-bound):
  Phase A/B (fused, per tile): stream K as natural (128, 2048) fp32
    tiles (two 2048-column chunks stacked on partition halves -> full
    DMA width). The first RES tiles stay resident in SBUF (reused by
    phase C with no re-read). Each tile is cast to a transient bf16
    copy on GpSimd, transposed 128 columns at a time on the PE
    (transpose-mode, bf16 -> bf16 PSUM), copied back to SBUF by DVE in
    (128, 4*64) groups, and consumed by paired Gram matmuls:
    lhsT = rhs = (128, 128) bf16 covering two K^T chunks; the two
    (64, 64) diagonal blocks of the (128, 128) PSUM accumulator hold
    the Gram sums (off-diagonal cross blocks are ignored). bf16 Gram is
    safe: sigmoid(G@G) saturates to exact 0/1; measured absmax output
    impact is ~5e-4.
  Middle: combine the diagonal blocks (one tiny SBUF->SBUF DMA to
    realign partitions), m3 = G@G duplicated into both PSUM partition
    halves, sigmoid on ACT, scale by gamma.
  Phase C: resident tiles compute immediately; the rest re-read.
    Per-half matmul against the gamma-scaled affinity (concurrent PE
    quadrants (0,0)/(64,64)), epilogue out = W_psum + x_tile in a
    single DVE add, store on the ACT HWDGE ring.
"""

from contextlib import ExitStack

import numpy as np

import concourse.bass as bass
import concourse.mybir as mybir
import concourse.tile as tile
from concourse.bass_utils import run_bass_kernel_spmd
from concourse.masks import make_identity


def split_waits(nc, max_waits: int = 1) -> int:
    """Walrus workaround: cayman instruction structs have a single
    NEURON_ISA_TPB_EVENTS slot and this walrus build rejects BIR
    instructions carrying more sync waits ("Too many sync wait commands").
    Move excess waits onto no-fuse NoOps inserted immediately before the
    instruction on the same engine — per-engine program order preserves the
    blocking semantics. Updates are never moved."""
    n_inserted = 0
    for f in nc.m.functions:
        for b in f.blocks:
            out = []
            changed = False
            for inst in b.instructions:
                si = inst.sync_info
                waits = list(si.on_wait) if si and si.on_wait else []
                if len(waits) > max_waits:
                    changed = True
                    si.on_wait = waits[:max_waits]
                    rest = waits[max_waits:]
                    for k in range(0, len(rest), max_waits):
                        n_inserted += 1
                        nop = mybir.InstNoOp(
                            name=f"I-waitsplit-{n_inserted}",
                            engine=inst.engine,
                            ins=[],
                            outs=[],
                            bass_nofuse=True,
                            sync_info=mybir.SyncInfo(
                                on_wait=rest[k : k + max_waits], on_update=[]
                            ),
                        )
                        nc.register_instruction(nop, overwrite=True)
                        out.append(nop)
                out.append(inst)
            if changed:
                b.instructions = out
    return n_inserted


F32 = mybir.dt.float32
BF16 = mybir.dt.bfloat16

B = 8
C = 64
N = 48 * 48 * 48  # 110592
P = 128

NT = 2048  # columns per stacked half; load tile = (128, NT) fp32 = 1 MiB
MM_N = 512  # fp32 moving-operand max free dim
RES = 16  # fp32 tiles kept resident for phase C (no re-read)


def build_channel_attention(n: int = N) -> bass.Bass:
    n_tiles = n // (2 * NT)
    assert n_tiles * 2 * NT == n
    n_res = min(RES, n_tiles)

    nc = bass.Bass()
    x_d = nc.dram_tensor("x", [C, n], F32, kind="ExternalInput")
    g_d = nc.dram_tensor("gamma", [1, 1], F32, kind="ExternalInput")
    o_d = nc.dram_tensor("out", [C, n], F32, kind="ExternalOutput")

    with tile.TileContext(nc) as tc, ExitStack() as ctx:
        singles = ctx.enter_context(tc.tile_pool(name="singles", bufs=1))
        res = ctx.enter_context(tc.tile_pool(name="res", bufs=max(n_res, 1)))
        xin = ctx.enter_context(tc.tile_pool(name="xin", bufs=4))
        rkp = ctx.enter_context(tc.tile_pool(name="rkp", bufs=3))
        trsb = ctx.enter_context(tc.tile_pool(name="trsb", bufs=4))
        oout = ctx.enter_context(tc.tile_pool(name="oout", bufs=3))
        pstr = ctx.enter_context(tc.tile_pool(name="pstr", bufs=3, space="PSUM"))
        gram_ps = ctx.enter_context(tc.tile_pool(name="gram_ps", bufs=1, space="PSUM"))
        w_ps_pool = ctx.enter_context(tc.tile_pool(name="w_ps", bufs=3, space="PSUM"))

        # constants: gamma broadcast, identity for PE transposes (both halves)
        gamma_sb = singles.tile([P, 1], F32)
        nc.gpsimd.dma_start(
            out=gamma_sb,
            in_=bass.AP(tensor=g_d, offset=0, ap=[[0, P], [1, 1]]),
        )
        ident = singles.tile([P, C], BF16)
        make_identity(nc, ident[0:C, :])
        make_identity(nc, ident[C:P, :])

        xn = x_d[:, :].rearrange("c (t f) -> t c f", f=NT)
        on = o_d[:, :].rearrange("c (t f) -> t c f", f=NT)

        # ---------------- Phase A+B: load (+cast) + PE-transpose Gram -------
        gram = gram_ps.tile([P, P], F32)  # two (64,64) Gram halves on diagonal
        x_tiles = []
        mm_i = 0
        n_gram_mms = n_tiles * 2 * (NT // 256)
        for t in range(n_tiles):
            if t < n_res:
                xt = res.tile([P, NT], F32, tag="res")
            else:
                xt = xin.tile([P, NT], F32, tag="xin")
            nc.sync.dma_start(out=xt, in_=xn[2 * t : 2 * t + 2])
            x_tiles.append(xt)
            rkt = rkp.tile([P, NT], BF16, tag="rkp")
            nc.gpsimd.tensor_copy(rkt, xt)
            for a in range(2):
                for g in range(NT // 512):  # groups of 4 transposed chunks
                    ps = pstr.tile([P, 4, C], BF16, tag="pstr")
                    for j in range(4):
                        ch = (4 * g + j) * 128
                        nc.tensor.transpose(
                            ps[:, j, :],
                            rkt[a * C : (a + 1) * C, ch : ch + 128],
                            ident[a * C : (a + 1) * C, :],
                            tile_position=(a * C, 0),
                        )
                    tsb = trsb.tile([P, 4, C], BF16, tag="trsb")
                    nc.vector.tensor_copy(tsb, ps)
                    for h in range(2):
                        nc.tensor.matmul(
                            gram,
                            tsb[:, 2 * h : 2 * h + 2, :],
                            tsb[:, 2 * h : 2 * h + 2, :],
                            start=(mm_i == 0),
                            stop=(mm_i == n_gram_mms - 1),
                        )
                        mm_i += 1
        assert mm_i == n_gram_mms

        # ---------------- Middle: A = gamma * sigmoid(G @ G) ----------------
        gsb2 = singles.tile([P, P], F32)
        nc.vector.tensor_copy(gsb2, gram)
        godd = singles.tile([C, C], F32)
        nc.gpsimd.dma_start(out=godd, in_=gsb2[C:P, C:P])
        g_sb = singles.tile([C, C], F32)
        nc.vector.tensor_add(g_sb, gsb2[0:C, 0:C], godd)
        m3 = gram_ps.tile([P, C], F32)
        nc.tensor.matmul(m3[0:C, :], g_sb, g_sb, start=True, stop=True)
        nc.tensor.matmul(m3[C:P, :], g_sb, g_sb, start=True, stop=True)
        a2 = singles.tile([P, C], F32)
        zero_bias = singles.tile([P, 1], F32)
        nc.vector.memset(zero_bias, 0.0)
        nc.scalar.activation(
            a2, m3, mybir.ActivationFunctionType.Sigmoid, bias=zero_bias
        )
        nc.vector.tensor_scalar_mul(a2, a2, gamma_sb)

        # ---------------- Phase C: out = (gamma*A) @ K + x ----------------
        # issue the streamed re-reads first so DMA starts immediately
        for t in range(n_res, n_tiles):
            xt = xin.tile([P, NT], F32, tag="xin")
            nc.sync.dma_start(out=xt, in_=xn[2 * t : 2 * t + 2])
            x_tiles[t] = xt

        def tail(t):
            xt = x_tiles[t]
            ot = oout.tile([P, NT], F32, tag="oout")
            for f0 in range(0, NT, MM_N):
                w_ps = w_ps_pool.tile([P, MM_N], F32, tag="w_ps")
                nc.tensor.matmul(
                    w_ps[0:C, :],
                    a2[0:C, :],
                    xt[0:C, f0 : f0 + MM_N],
                    start=True,
                    stop=True,
                )
                nc.tensor.matmul(
                    w_ps[C:P, :],
                    a2[C:P, :],
                    xt[C:P, f0 : f0 + MM_N],
                    start=True,
                    stop=True,
                )
                nc.vector.tensor_add(
                    ot[:, f0 : f0 + MM_N], w_ps, xt[:, f0 : f0 + MM_N]
                )
            nc.scalar.dma_start(out=on[2 * t : 2 * t + 2], in_=ot)

        for t in range(n_res):  # resident tiles: no load dependency
            tail(t)
        for t in range(n_res, n_tiles):
            tail(t)

    split_waits(nc)
    return nc


_NC_CACHE: dict[int, bass.Bass] = {}


def _get_nc(n: int = N) -> bass.Bass:
    if n not in _NC_CACHE:
        _NC_CACHE[n] = build_channel_attention(n)
    return _NC_CACHE[n]


def kernel(x: np.ndarray, gamma: np.ndarray, **run_kwargs):
    x = np.ascontiguousarray(np.asarray(x, dtype=np.float32))
    b = x.shape[0]
    n = int(np.prod(x.shape[1:])) // C
    g11 = np.asarray(gamma, dtype=np.float32).reshape(1, 1)
    xs = x.reshape(b, C, n)

    nc = _get_nc(n)
    in_maps = [{"x": xs[i], "gamma": g11} for i in range(b)]
    res = run_bass_kernel_spmd(nc, in_maps, core_ids=list(range(b)), **run_kwargs)
    out = np.stack([res.results[i]["out"] for i in range(b)])
    if run_kwargs.get("trace"):
        kernel.last_result = res
    return out.reshape(x.shape).astype(np.float32)
